# revision 10
# baseline (speedup 1.0000x reference)
"""MoE top-2 routed FFN (B=4, S=2048, D=1024, H=2048, E=8) on 8 TRN2 NeuronCores.

Strategy (expert-parallel, matching the sharding hint):
  - Host computes the tiny gate (softmax top-2) and builds per-expert token
    lists ("all-to-all dispatch" done at the sharding step).
  - Core e receives the tokens routed to expert e (gathered, transposed,
    zero-padded to capacity C), plus expert e's weights pre-packed into the
    exact tile layouts the kernel consumes.
  - Each core runs a dense FFN  out = coef * ((relu(x@W1.T)^2 * (x@W3.T)) @ W2.T)
    over its C tokens.  Matmuls run bf16 with fp32 PSUM accumulation; the
    output is stored bf16 (end-to-end rel err ~6e-3 vs the 2e-2 budget),
    halving output DMA traffic on the single hardware DMA queue.
  - Host scatter-adds the per-expert outputs back ("combine").

Per-core kernel structure (single weight read; all DMAs on the sync queue,
which is the only fast path — per-engine queues start late and share the same
AXI port):
  warmup: a few matmuls on a memset tile bridge the DMA-queue ramp (the first
    ~2 MB arrives at only ~100-150 GB/s) and flip the HAM clock gate to
    2.4 GHz before real work starts.
  phase 1 (m-outer): for each H-tile m, gT[m] = relu(W1m@xT)^2 * (W3m@xT) over
    5 token groups.  x arrives as three contiguous 1024/128-column tensors
    (2 KB DMA rows keep the queue fast; tile granularity matches m=0's
    consumption order).  m=0 runs groups singly (paced against the x stream);
    m>=1 runs a pair + triple sharing each k-loop.
  phase 2: for each 128-token tile: out[tok, :] = (gT.T @ W2T) * coef, one
    merged [128, 1024] bf16 DMA per tile (2 KB rows drain fastest), on a
    by-then-idle DMA queue.
"""

import os
import sys

import numpy as np

if os.path.isdir("/opt/trn_rl_repo") and "/opt/trn_rl_repo" not in sys.path:
    sys.path.insert(0, "/opt/trn_rl_repo")

import ml_dtypes

import concourse.bacc as bacc
import concourse.mybir as mybir
from concourse.bass_utils import run_bass_kernel_spmd
from concourse.tile import TileContext

B, S, D, H, E = 4, 2048, 1024, 2048, 8
N = B * S
P = 128
KT = D // P   # 8 contraction tiles over D
MT = H // P   # 16 tiles over H

F32 = mybir.dt.float32
BF16 = mybir.dt.bfloat16
BF16_NP = ml_dtypes.bfloat16

# Set by test harness to capture profiling info.
TRACE = False
LAST_RESULTS = None


def build_kernel(C):
    TT = C // P
    nc = bacc.Bacc("TRN2", target_bir_lowering=False)

    xa_d = nc.dram_tensor("xa", [KT, P, 1024], BF16, kind="ExternalInput")
    xb_d = nc.dram_tensor("xb", [KT, P, 1024], BF16, kind="ExternalInput")
    xc_d = nc.dram_tensor("xc", [KT, P, C - 2048], BF16, kind="ExternalInput")
    w1p = nc.dram_tensor("w1p", [MT, P, KT * P], BF16, kind="ExternalInput")
    w3p = nc.dram_tensor("w3p", [MT, P, KT * P], BF16, kind="ExternalInput")
    w2p = nc.dram_tensor("w2p", [MT, P, D], BF16, kind="ExternalInput")
    cf = nc.dram_tensor("cf", [P, TT], F32, kind="ExternalInput")
    out = nc.dram_tensor("out", [TT, P, D], BF16, kind="ExternalOutput")

    groups = [(0, 512), (512, 512), (1024, 512), (1536, 512), (2048, C - 2048)]
    assert C - 2048 in range(1, 513)
    NG = len(groups)

    with TileContext(nc) as tc:
        with (
            tc.tile_pool(name="x_pool", bufs=1) as x_pool,
            tc.tile_pool(name="g_pool", bufs=1) as g_pool,
            tc.tile_pool(name="w13_pool", bufs=4) as w13_pool,
            tc.tile_pool(name="w2_pool", bufs=MT) as w2_pool,
            tc.tile_pool(name="tmp_pool", bufs=3) as tmp_pool,
            tc.tile_pool(name="ob_pool", bufs=2) as ob_pool,
            tc.tile_pool(name="const_pool", bufs=1) as const_pool,
            tc.tile_pool(name="psAB", bufs=3, space="PSUM") as psAB_pool,
            tc.tile_pool(name="psO", bufs=2, space="PSUM") as psO_pool,
        ):
            # --- PE warmup: flip the HAM clock gate (1.2->2.4GHz) while the
            # DMA queue ramps.  Depends only on a local memset, so it starts
            # as soon as the PE sequencer is live. ---------------------------
            warm = const_pool.tile([P, 512], BF16, tag="warm")
            nc.any.memset(warm[:], 0.0)
            pswarm = psO_pool.tile([P, 512], F32, tag="psO", name="pswarm")
            for i in range(9):
                nc.tensor.matmul(pswarm[:], warm[:, :P], warm[:],
                                 start=(i == 0), stop=(i == 8))
            warmsink = const_pool.tile([P, 1], F32, tag="warmsink")
            nc.vector.tensor_scalar_mul(warmsink[:], pswarm[:, :1], 0.0)

            # DMA emission order IS queue order: w1[m0] first, then x groups
            # in consumption order interleaved with w3[m0].
            w1t0 = w13_pool.tile([P, KT * P], BF16, tag="w1t", name="w1_0")
            nc.sync.dma_start(w1t0[:], w1p[0])
            xtiles = {"a": [], "b": [], "c": []}

            def emit_x(key, dram, w):
                for k in range(KT):
                    t = x_pool.tile([P, w], BF16, tag=f"x{key}{k}",
                                    name=f"x_{key}_{k}")
                    nc.sync.dma_start(t[:], dram[k])
                    xtiles[key].append(t)

            emit_x("a", xa_d, 1024)
            w3t0 = w13_pool.tile([P, KT * P], BF16, tag="w3t", name="w3_0")
            nc.sync.dma_start(w3t0[:], w3p[0])
            emit_x("b", xb_d, 1024)
            emit_x("c", xc_d, C - 2048)

            def xslice(gi, k):
                if gi < 4:
                    t = xtiles["ab"[gi // 2]][k]
                    off = (gi % 2) * 512
                    return t[:, off:off + 512]
                return xtiles["c"][k][:]

            gts = []
            for m in range(MT):
                gts.append(g_pool.tile([P, C], BF16, tag=f"g{m}",
                                       name=f"g_{m}"))

            def do_block(m, w1t, w3t, blk):
                # one k-loop over 1-3 token groups (gi, g0, gw)
                psAs, psBs = [], []
                for (gi, g0, gw) in blk:
                    psAs.append(psAB_pool.tile(
                        [P, 512], F32, tag="psA", name=f"psA_{m}_{g0}"))
                    psBs.append(psAB_pool.tile(
                        [P, 512], F32, tag="psB", name=f"psB_{m}_{g0}"))
                for k in range(KT):
                    for (gi, g0, gw), ps in zip(blk, psAs):
                        nc.tensor.matmul(
                            ps[:, :gw], w1t[:, k * P:(k + 1) * P],
                            xslice(gi, k),
                            start=(k == 0), stop=(k == KT - 1),
                        )
                for k in range(KT):
                    for (gi, g0, gw), ps in zip(blk, psBs):
                        nc.tensor.matmul(
                            ps[:, :gw], w3t[:, k * P:(k + 1) * P],
                            xslice(gi, k),
                            start=(k == 0), stop=(k == KT - 1),
                        )
                for (gi, g0, gw), psA, psB in zip(blk, psAs, psBs):
                    r = tmp_pool.tile([P, 512], BF16, tag="r",
                                      name=f"r_{m}_{g0}")
                    nc.vector.tensor_relu(r[:, :gw], psA[:, :gw])
                    t2 = tmp_pool.tile([P, 512], BF16, tag="t2",
                                       name=f"t2_{m}_{g0}")
                    nc.vector.tensor_mul(t2[:, :gw], r[:, :gw], r[:, :gw])
                    nc.vector.tensor_mul(
                        gts[m][:, g0:g0 + gw], t2[:, :gw], psB[:, :gw])

            blocks = [(gi, g0, gw) for gi, (g0, gw) in enumerate(groups)]
            w2ts = []
            cft = None
            for m in range(MT):
                if m == 0:
                    w1t, w3t = w1t0, w3t0
                else:
                    w1t = w13_pool.tile([P, KT * P], BF16, tag="w1t",
                                        name=f"w1_{m}")
                    nc.sync.dma_start(w1t[:], w1p[m])
                    w3t = w13_pool.tile([P, KT * P], BF16, tag="w3t",
                                        name=f"w3_{m}")
                    nc.sync.dma_start(w3t[:], w3p[m])
                if m == 0:
                    # singles: paces the x stream during the ramp window
                    for blk in blocks:
                        do_block(m, w1t, w3t, [blk])
                else:
                    do_block(m, w1t, w3t, blocks[0:2])
                    do_block(m, w1t, w3t, blocks[2:5])
                if m == 2:
                    # W2 + coef ride the queue behind phase-1 inputs, long
                    # before phase 2 needs them.
                    cft = const_pool.tile([P, TT], F32, tag="cft")
                    nc.sync.dma_start(cft[:], cf[:])
                    for hk in range(MT):
                        w2t = w2_pool.tile([P, D], BF16, tag="w2t",
                                           name=f"w2_{hk}")
                        nc.sync.dma_start(w2t[:], w2p[hk])
                        w2ts.append(w2t)

            # --- phase 2: out[tok, d] = coef * (g.T @ W2T) ------------------
            for t in range(TT):
                ob = ob_pool.tile([P, D], BF16, tag="ob", name=f"ob_{t}")
                for dg in range(2):
                    pso = psO_pool.tile([P, 512], F32, tag="psO",
                                        name=f"psO_{t}_{dg}")
                    for hk in range(MT):
                        nc.tensor.matmul(
                            pso[:],
                            gts[hk][:, t * P:(t + 1) * P],
                            w2ts[hk][:, dg * 512:(dg + 1) * 512],
                            start=(hk == 0), stop=(hk == MT - 1),
                        )
                    nc.vector.tensor_scalar_mul(
                        ob[:, dg * 512:(dg + 1) * 512], pso[:],
                        cft[:, t:t + 1])
                nc.sync.dma_start(out[t], ob[:])

    if not nc.is_finalized():
        nc.finalize()
    return nc


def kernel(x, W1, W2, W3, gate_w, gate_b):
    global LAST_RESULTS

    xf = np.ascontiguousarray(x.reshape(N, D).astype(np.float32, copy=False))

    # ---- gate: softmax + top-2 (tiny, done on host) ------------------------
    logits = xf @ gate_w.T.astype(np.float32) + gate_b.astype(np.float32)
    logits -= logits.max(axis=-1, keepdims=True)
    probs = np.exp(logits)
    probs /= probs.sum(axis=-1, keepdims=True)
    order = np.argsort(-probs, axis=-1, kind="stable")
    i1, i2 = order[:, 0], order[:, 1]
    ar = np.arange(N)
    p1, p2 = probs[ar, i1], probs[ar, i2]
    ps = p1 + p2
    c1, c2 = p1 / ps, p2 / ps

    idx_list, coef_list = [], []
    for e in range(E):
        m1 = i1 == e
        m2 = i2 == e
        ide = np.nonzero(m1 | m2)[0]
        ce = np.where(m1[ide], c1[ide], c2[ide]).astype(np.float32)
        idx_list.append(ide)
        coef_list.append(ce)

    nmax = max(len(i) for i in idx_list)
    C = max(((nmax + P - 1) // P) * P, 2048 + P)
    TT = C // P

    # ---- per-core input packing -------------------------------------------
    in_maps = []
    for e in range(E):
        ide, ce = idx_list[e], coef_list[e]
        ne = len(ide)

        xg = np.zeros((C, D), np.float32)
        xg[:ne] = xf[ide]
        xt_np = np.ascontiguousarray(xg.T).reshape(KT, P, C).astype(BF16_NP)
        xa_np = np.ascontiguousarray(xt_np[:, :, :1024])
        xb_np = np.ascontiguousarray(xt_np[:, :, 1024:2048])
        xc_np = np.ascontiguousarray(xt_np[:, :, 2048:])

        w1e = np.asarray(W1[e], np.float32)  # [H, D]
        w3e = np.asarray(W3[e], np.float32)  # [H, D]
        w2e = np.asarray(W2[e], np.float32)  # [D, H]
        # [m, h, k, d] -> [m, d, k, h] : packed[m][d, k*128+h] = W1[m*128+h, k*128+d]
        w1p_np = np.ascontiguousarray(
            w1e.reshape(MT, P, KT, P).transpose(0, 3, 2, 1)
        ).reshape(MT, P, KT * P).astype(BF16_NP)
        w3p_np = np.ascontiguousarray(
            w3e.reshape(MT, P, KT, P).transpose(0, 3, 2, 1)
        ).reshape(MT, P, KT * P).astype(BF16_NP)
        # W2T[h, d] tiles: [hk, h, d]
        w2p_np = np.ascontiguousarray(w2e.T).reshape(MT, P, D).astype(BF16_NP)

        cfe = np.zeros(C, np.float32)
        cfe[:ne] = ce
        cf_np = np.ascontiguousarray(cfe.reshape(TT, P).T)

        in_maps.append(
            {"xa": xa_np, "xb": xb_np, "xc": xc_np, "w1p": w1p_np,
             "w3p": w3p_np, "w2p": w2p_np, "cf": cf_np}
        )

    # ---- build + run on 8 cores -------------------------------------------
    nc = build_kernel(C)
    res = None
    last_exc = None
    for attempt in range(3):
        try:
            res = run_bass_kernel_spmd(
                nc, in_maps, core_ids=list(range(E)),
                trace=TRACE and attempt == 0,
            )
            break
        except Exception as exc:  # transient device wedge / trace plumbing
            last_exc = exc
    if res is None:
        raise last_exc
    LAST_RESULTS = res

    # ---- combine ----------------------------------------------------------
    out = np.zeros((N, D), np.float32)
    for e in range(E):
        ide = idx_list[e]
        oe = res.results[e]["out"]  # [TT, P, D] bf16
        oe = oe.astype(np.float32).reshape(C, D)
        out[ide] += oe[: len(ide)]

    return out.reshape(B, S, D)
